# revision 1
# baseline (speedup 1.0000x reference)
"""DisMax loss first part: logits = -(|s|*d + mean_c(|s|*d)) / temp, where
d[b,c] = ||fn_b - pn_c|| / sqrt(2) = sqrt(1 - cos(f_b, p_c)) for l2-normalized rows.

Strategy: data-parallel over the batch across 8 NeuronCores. Each core:
  [1024, 512] features x [512, 10000] transposed prototypes -> [1024, 10000].
Prototypes are passed host-transposed (layout prep only) so the device GEMM
operands are already [d, .]; the device computes all numerics:
  - prototype class norms: DVE/GPSIMD squares -> PE ones-matmul column
    sums -> one ACT rsqrt per chunk pair -> PE partition-broadcast matmul ->
    fused DVE normalize+cast (f32 staging x inv-norm -> bf16 pnT);
  - feature row norms: DVE square+row-sum, ACT rsqrt, DVE scale-cast,
    PE transposes into fnT;
  - main GEMM: bf16, fp32 PSUM (2-bank tiles), [128 x 1000] chunks; ACT
    computes sqrt(1 - cos) out of PSUM with fused row-sum accumulation;
  - GPSIMD applies out = dist*c0 + rowsum*c1 (c0 = -|scale|/temp,
    c1 = c0/10000); 1 MB chunked DMAs stream the 40 MB result to HBM.
"""

import sys
import types

for _p in ("/opt/trn_rl_repo", "/root/.axon_site"):
    if _p not in sys.path:
        sys.path.insert(0, _p)

# The NTFF profiling hook module is absent from this image's antenv package;
# inject the ctypes-based equivalent so trace=True works when requested.
if "antenv.axon_hooks" not in sys.modules:
    try:
        import trn_agent_boot.trn_boot as _tb

        _hook = _tb._ntff_profile_via_ctypes("/opt/axon/libaxon_pjrt.so")
        _m = types.ModuleType("antenv.axon_hooks")
        _m.get_axon_ntff_profile_hook = lambda: _hook
        sys.modules["antenv.axon_hooks"] = _m
    except Exception:
        pass

import numpy as np

import concourse.bacc as bacc
import concourse.tile as tile
import concourse.mybir as mybir
from concourse.bass_utils import run_bass_kernel_spmd

F32 = mybir.dt.float32
BF16 = mybir.dt.bfloat16
ALU = mybir.AluOpType
ACTF = mybir.ActivationFunctionType

N_CORES = 8
B, C, D = 8192, 10000, 512
BPC = B // N_CORES          # 1024 batch rows per core
NB = BPC // 128             # 8 batch tiles
ND = D // 128               # 4 contraction tiles
CCH = 500                   # matmul free-dim chunk (fits 1 PSUM bank in f32)
BCH = 1000                  # ACT/sqrt + prototype-prep chunk
NBCH = C // BCH             # 10 chunks
P2 = 2000                   # pass-2 / store chunk (1 MB DMA)
NP2 = C // P2               # 5
FG = 1                      # feature tiles per staged load (256 KB DMA)


def build_nc():
    nc = bacc.Bacc("TRN2", target_bir_lowering=False, debug=False,
                   num_devices=N_CORES)
    f_h = nc.dram_tensor("f", [BPC, D], F32, kind="ExternalInput")
    pt_h = nc.dram_tensor("pt", [D, C], F32, kind="ExternalInput")
    s_h = nc.dram_tensor("s", [1, 2], F32, kind="ExternalInput")
    o_h = nc.dram_tensor("o", [BPC, C], F32, kind="ExternalOutput")

    from contextlib import ExitStack

    with tile.TileContext(nc) as tc:
        with ExitStack() as stack:
            ep = stack.enter_context
            const_pool = ep(tc.tile_pool(name="const", bufs=1))
            persist_pool = ep(tc.tile_pool(name="persist", bufs=1))
            fstage_pool = ep(tc.tile_pool(name="fstage", bufs=2))
            pstage_pool = ep(tc.tile_pool(name="pstage", bufs=4))
            rows_pool = ep(tc.tile_pool(name="rows", bufs=1))
            bf_pool = ep(tc.tile_pool(name="bfc", bufs=2))
            sq_pool = ep(tc.tile_pool(name="sq", bufs=4))
            norm_pool = ep(tc.tile_pool(name="norms", bufs=2))
            invb_pool = ep(tc.tile_pool(name="invb", bufs=2))
            dist_pool = ep(tc.tile_pool(name="dist", bufs=6))
            rs_pool = ep(tc.tile_pool(name="rs", bufs=2))
            psum_c_pool = ep(tc.tile_pool(name="ps_c", bufs=3, space="PSUM"))
            psum_m_pool = ep(tc.tile_pool(name="ps_m", bufs=2, space="PSUM"))
            # persistent bf16 transposed operands
            pnT = persist_pool.tile([128, ND, C], BF16, tag="pnT")     # 78 KB/p
            fnT = persist_pool.tile([128, ND, BPC], BF16, tag="fnT")   # 8 KB/p
            cb = persist_pool.tile([128, 2], F32, tag="cb")            # c0, c1

            ones_f = const_pool.tile([1, 128], F32, tag="ones_f")
            nc.vector.memset(ones_f[:, :], 1.0)
            ones_b = const_pool.tile([128, 1], BF16, tag="ones_b")
            nc.vector.memset(ones_b[:, :], 1.0)
            from concourse import masks

            ident = const_pool.tile([128, 128], BF16, tag="ident")
            masks.make_identity(nc, ident[:, :])

            # ---- scalar params: c0 = -|ds|/temp, c1 = c0/C ----------------
            stile = const_pool.tile([1, 2], F32, tag="stile")
            nc.sync.dma_start(stile[:, :], s_h[:, :])
            cv = const_pool.tile([1, 2], F32, tag="cvals")
            tmp = const_pool.tile([1, 2], F32, tag="scaltmp")
            nc.scalar.activation(tmp[:, 0:1], stile[:, 0:1], ACTF.Abs)
            nc.vector.reciprocal(tmp[:, 1:2], stile[:, 1:2])
            nc.vector.scalar_tensor_tensor(cv[:, 0:1], tmp[:, 0:1], -1.0,
                                           tmp[:, 1:2], op0=ALU.mult,
                                           op1=ALU.mult)
            nc.vector.tensor_scalar(cv[:, 1:2], cv[:, 0:1], 1.0 / C, None,
                                    op0=ALU.mult)
            ps_b = psum_m_pool.tile([128, CCH], F32, tag="m")
            nc.tensor.matmul(ps_b[:, :2], ones_f[:, :], cv[:, :], start=True,
                             stop=True)
            nc.vector.tensor_copy(cb[:, :], ps_b[:, :2])

            # ---- feature prep ---------------------------------------------
            f_r = f_h[:, :].rearrange("(g t p) d -> g p t d", p=128, t=FG)
            for g in range(NB // FG):
                fst = fstage_pool.tile([128, FG, D], F32, tag="fst")
                nc.sync.dma_start(fst[:, :, :], f_r[g])
                fss = norm_pool.tile([128, FG], F32, tag="fss")
                finv = norm_pool.tile([128, FG], F32, tag="finv")
                for t in range(FG):
                    fsq = bf_pool.tile([128, D], BF16, tag="bfc", name=f"fsq_{g}_{t}")
                    nc.vector.scalar_tensor_tensor(
                        fsq[:, :], fst[:, t, :], 1.0, fst[:, t, :],
                        op0=ALU.mult, op1=ALU.mult,
                        accum_out=fss[:, t:t + 1])
                nc.scalar.activation(finv[:, :], fss[:, :],
                                     ACTF.Abs_reciprocal_sqrt)
                for t in range(FG):
                    i = g * FG + t
                    fbf = bf_pool.tile([128, D], BF16, tag="bfc")
                    nc.vector.tensor_scalar(fbf[:, :], fst[:, t, :],
                                            finv[:, t:t + 1], None,
                                            op0=ALU.mult)
                    ps_t0 = psum_m_pool.tile([128, CCH], F32, tag="m", name="ps_t0")
                    ps_t = ps_t0[:, :].bitcast(BF16)[:, :ND * 128].rearrange("p (d c) -> p d c", d=ND)
                    for d in range(ND):
                        nc.tensor.transpose(ps_t[:, d, :],
                                            fbf[:, d * 128:(d + 1) * 128],
                                            ident[:, :])
                    nc.vector.tensor_copy(
                        fnT[:, :, i * 128:(i + 1) * 128], ps_t[:, :, :])

            # ---- prototype prep (host-transposed pT in DRAM) ---------------
            # processed in pairs of 1000-class chunks so the ACT rsqrt (and
            # its activation-table load) runs once per pair
            pt_r = pt_h[:, :].rearrange("(t p) c -> p t c", p=128)
            for pp in range(NBCH // 2):
                pair_psts = []
                srow = rows_pool.tile([1, 2 * BCH], F32, tag="srow")
                irow = srow
                for ci in range(2):
                    cc = 2 * pp + ci
                    c0, c1 = cc * BCH, (cc + 1) * BCH
                    psts = []
                    sqs = []
                    for h in range(2):
                        pst = pstage_pool.tile([128, 2, BCH], F32, tag="pstg",
                                               name=f"pst_{cc}_{h}")
                        psts.append(pst)
                        sqh = sq_pool.tile([128, 2, BCH], BF16, tag="sq",
                                           name=f"sq_{cc}_{h}")
                        sqs.append(sqh)
                        nc.sync.dma_start(pst[:, :, :],
                                          pt_r[:, 2 * h:2 * h + 2, c0:c1])
                        for hh in range(2):
                            # squares for the class-norm column sums, split
                            # DVE/GPSIMD (not ACT: keeps the table stable)
                            sqeng = (nc.gpsimd if (h == 0 and hh == 0)
                                     else nc.vector)
                            sqeng.tensor_tensor(sqh[:, hh, :], pst[:, hh, :],
                                                pst[:, hh, :], op=ALU.mult)
                    pair_psts.append(psts)
                    for sub in range(2):
                        ss0 = psum_m_pool.tile([128, CCH], F32, tag="m",
                                               name="ss0")
                        ss = ss0[:1, :]
                        for d in range(ND):
                            nc.tensor.matmul(
                                ss[:, :], ones_b[:, :],
                                sqs[d // 2][:, d % 2,
                                            sub * CCH:(sub + 1) * CCH],
                                start=(d == 0), stop=(d == ND - 1))
                        off = ci * BCH + sub * CCH
                        nc.vector.tensor_copy(srow[:, off:off + CCH],
                                              ss[:, :])
                # 1/norm in a single ACT op per pair
                nc.scalar.activation(irow[:, :], srow[:, :],
                                     ACTF.Abs_reciprocal_sqrt)
                for ci in range(2):
                    cc = 2 * pp + ci
                    c0, c1 = cc * BCH, (cc + 1) * BCH
                    ib = invb_pool.tile([128, BCH], F32, tag="invb")
                    for sub in range(2):
                        bc = psum_m_pool.tile([128, CCH], F32, tag="m")
                        off = ci * BCH + sub * CCH
                        nc.tensor.matmul(bc[:, :], ones_f[:, :],
                                         irow[:, off:off + CCH],
                                         start=True, stop=True)
                        nc.vector.tensor_copy(
                            ib[:, sub * CCH:(sub + 1) * CCH], bc[:, :])
                    # fused normalize + cast: pnT = pst * (1/||p_c||)
                    for d in range(ND):
                        nc.vector.tensor_tensor(
                            pnT[:, d, c0:c1],
                            pair_psts[ci][d // 2][:, d % 2, :],
                            ib[:, :], op=ALU.mult)

            # ---- main loop -------------------------------------------------
            for i in range(NB):
                rs = rs_pool.tile([128, NBCH], F32, tag="rs")
                dqs = []
                for q in range(NP2):
                    dq = dist_pool.tile([128, P2], F32, tag="dist")
                    dqs.append(dq)
                    for k in range(P2 // BCH):
                        bc = q * (P2 // BCH) + k           # 1000-chunk index
                        pc = psum_c_pool.tile([128, 2, 512], F32, tag="pc")
                        for h in range(2):
                            c0 = bc * BCH + h * CCH
                            for d in range(ND):
                                nc.tensor.matmul(
                                    pc[:, h, :CCH],
                                    fnT[:, d, i * 128:(i + 1) * 128],
                                    pnT[:, d, c0:c0 + CCH],
                                    start=(d == 0), stop=(d == ND - 1))
                        # dist = sqrt(1 - cos); accum_out = row-chunk sum
                        dv = dq[:, k * BCH:(k + 1) * BCH].rearrange(
                            "p (h c) -> p h c", h=2)
                        nc.scalar.activation(
                            dv, pc[:, :, :CCH],
                            ACTF.Sqrt, bias=1.0, scale=-1.0,
                            accum_out=rs[:, bc:bc + 1])
                rsum = norm_pool.tile([128, 1], F32, tag="rsum")
                bvec = norm_pool.tile([128, 1], F32, tag="bvec")
                nc.vector.reduce_sum(rsum[:, :], rs[:, :],
                                     axis=mybir.AxisListType.X)
                nc.vector.tensor_scalar(bvec[:, :], rsum[:, :], cb[:, 1:2],
                                        None, op0=ALU.mult)
                for q in range(NP2):
                    ob0 = pstage_pool.tile([128, 2, BCH], F32, tag="pstg",
                                           name=f"ob_{i}_{q}")
                    ob = ob0[:, :, :].rearrange("p a b -> p (a b)")
                    nc.gpsimd.tensor_scalar(ob, dqs[q][:, :],
                                            cb[:, 0:1], bvec[:, 0:1],
                                            op0=ALU.mult, op1=ALU.add)
                    nc.sync.dma_start(
                        o_h[i * 128:(i + 1) * 128, q * P2:(q + 1) * P2],
                        ob)

    nc.compile()
    return nc


_CACHE = {}


def _get_nc():
    if "nc" not in _CACHE:
        _CACHE["nc"] = build_nc()
    return _CACHE["nc"]


def make_in_maps(features, prototypes, distance_scale, temperature):
    f = np.ascontiguousarray(np.asarray(features, dtype=np.float32))
    pt = np.ascontiguousarray(np.asarray(prototypes, dtype=np.float32).T)
    s = np.array([[np.float32(np.asarray(distance_scale).reshape(-1)[0]),
                   np.float32(np.asarray(temperature).reshape(-1)[0])]],
                 dtype=np.float32)
    return [
        {"f": f[i * BPC:(i + 1) * BPC], "pt": pt, "s": s}
        for i in range(N_CORES)
    ]


def run(features, prototypes, distance_scale, temperature, **kwargs):
    nc = _get_nc()
    in_maps = make_in_maps(features, prototypes, distance_scale, temperature)
    res = run_bass_kernel_spmd(nc, in_maps, core_ids=list(range(N_CORES)),
                               **kwargs)
    out = np.concatenate([res.results[i]["o"] for i in range(N_CORES)], axis=0)
    return out, res


def kernel(features, prototypes, distance_scale, temperature):
    out, _ = run(features, prototypes, distance_scale, temperature)
    return out



# revision 6
# speedup vs baseline: 1.4257x; 1.4257x over previous
"""DisMax loss first part — collective variant: prototype normalization is
sharded across the 8 cores (1250 classes each) and the fp8 operands are
exchanged with a 5 MB HBM AllGather, removing 7/8 of the prep DVE/DMA work.

Core pipeline (per core):
  - shard prep: one 2.5 MB DMA, bf16 squares (DVE/GPSIMD), ones[128,128]
    column-sum matmuls, one ACT Abs_reciprocal_sqrt, fused DVE
    normalize+fp8 cast (values x16);
  - AllGather via DRAM bounce buffers (NRT collective), readback into the
    full fp8 pnT [128, 4, 10000];
  - main loop: DoubleRow fp8 GEMM (psum = 256*cos), ACT sqrt(1 - psum/256)
    in bf16 with fused row-chunk sums, pass-2 (dist*c0 + rowsum*c1) split
    DVE/GPSIMD, bf16 stores; host does the exact widening cast to f32.
"""

import sys
import types

for _p in ("/opt/trn_rl_repo", "/root/.axon_site"):
    if _p not in sys.path:
        sys.path.insert(0, _p)

if "antenv.axon_hooks" not in sys.modules:
    try:
        import trn_agent_boot.trn_boot as _tb

        _hook = _tb._ntff_profile_via_ctypes("/opt/axon/libaxon_pjrt.so")
        _m = types.ModuleType("antenv.axon_hooks")
        _m.get_axon_ntff_profile_hook = lambda: _hook
        sys.modules["antenv.axon_hooks"] = _m
    except Exception:
        pass

import numpy as np

import concourse.bacc as bacc
import concourse.tile as tile
import concourse.mybir as mybir
from concourse.bass_utils import run_bass_kernel_spmd

F32 = mybir.dt.float32
BF16 = mybir.dt.bfloat16
FP8 = mybir.dt.float8e4
ALU = mybir.AluOpType
ACTF = mybir.ActivationFunctionType
DR = mybir.MatmulPerfMode.DoubleRow

N_CORES = 8
B, C, D = 8192, 10000, 512
BPC = B // N_CORES          # 1024 batch rows per core
NB = BPC // 128             # 8 batch tiles
ND = D // 128               # 4 contraction tiles (2 DoubleRow pairs)
CCH = 500                   # matmul free-dim chunk (fits 1 PSUM bank in f32)
GRP = 4                     # chunks per PSUM group (4 banks, 2000 classes)
NG = C // (CCH * GRP)       # 5 groups per row block
SH = C // N_CORES           # 1250 classes prepped per core
SCL = 16.0                  # fp8 operand scale; psum = 256*cos
ISCL2 = 1.0 / (SCL * SCL)


def build_nc():
    nc = bacc.Bacc("TRN2", target_bir_lowering=False, debug=False,
                   num_devices=N_CORES)
    f_h = nc.dram_tensor("f", [BPC, D], F32, kind="ExternalInput")
    pt_h = nc.dram_tensor("pt", [D, SH], F32, kind="ExternalInput")
    s_h = nc.dram_tensor("s", [1, 2], F32, kind="ExternalInput")
    o_h = nc.dram_tensor("o", [BPC, C], BF16, kind="ExternalOutput")

    from contextlib import ExitStack

    with tile.TileContext(nc) as tc:
        with ExitStack() as stack:
            ep = stack.enter_context
            const_pool = ep(tc.tile_pool(name="const", bufs=1))
            persist_pool = ep(tc.tile_pool(name="persist", bufs=1))
            fstage_pool = ep(tc.tile_pool(name="fstage", bufs=2))
            pstage_pool = ep(tc.tile_pool(name="pstage", bufs=1))
            bf_pool = ep(tc.tile_pool(name="bfc", bufs=2))
            sq_pool = ep(tc.tile_pool(name="sq", bufs=1))
            norm_pool = ep(tc.tile_pool(name="norms", bufs=2))
            inv_pool = ep(tc.tile_pool(name="invp", bufs=1))
            dist_pool = ep(tc.tile_pool(name="dist", bufs=2))
            ob_pool = ep(tc.tile_pool(name="obuf", bufs=4))
            rs_pool = ep(tc.tile_pool(name="rs", bufs=2))
            psum_pool = ep(tc.tile_pool(name="ps4", bufs=2, space="PSUM"))
            dram_pool = ep(tc.tile_pool(name="dram", bufs=1, space="DRAM"))

            # persistent fp8 transposed operands (values scaled by 16)
            pnT = persist_pool.tile([128, ND, C], FP8, tag="pnT")   # 40 KB/p
            fnT = persist_pool.tile([128, ND, BPC], FP8, tag="fnT") # 4 KB/p
            cb = persist_pool.tile([128, 2], F32, tag="cb")         # c0, c1

            ones_f = const_pool.tile([1, 128], F32, tag="ones_f")
            nc.vector.memset(ones_f[:, :], 1.0)
            ones128 = const_pool.tile([128, 128], BF16, tag="ones128")
            nc.vector.memset(ones128[:, :], 1.0)
            from concourse import masks

            ident = const_pool.tile([128, 128], BF16, tag="ident")
            masks.make_identity(nc, ident[:, :])

            # ---- shard prototype prep (this core's 1250 classes) -----------
            pt_r = pt_h[:, :].rearrange("(t p) c -> p t c", p=128)
            pst = pstage_pool.tile([128, ND, SH], F32, tag="pstg")
            nc.sync.dma_start(pst[:, :, :], pt_r)
            sq = sq_pool.tile([128, ND, SH], BF16, tag="sq")
            for d in range(ND):
                sqeng = nc.gpsimd if d == 0 else nc.vector
                sqeng.tensor_tensor(sq[:, d, :], pst[:, d, :], pst[:, d, :],
                                    op=ALU.mult)
            cs0 = psum_pool.tile([128, GRP, 512], F32, tag="ps4", name="cs")
            subs = [(0, 500), (500, 500), (1000, 250)]
            for si, (off, w) in enumerate(subs):
                for d in range(ND):
                    nc.tensor.matmul(cs0[:, si, :w], ones128[:, :],
                                     sq[:, d, off:off + w],
                                     start=(d == 0), stop=(d == ND - 1))
            inv16 = inv_pool.tile([128, SH], F32, tag="inv16")
            iv01 = inv16[:, 0:1000].rearrange("p (s c) -> p s c", s=2)
            nc.scalar.activation(iv01, cs0[:, 0:2, :500],
                                 ACTF.Abs_reciprocal_sqrt, scale=ISCL2)
            nc.scalar.activation(inv16[:, 1000:1250], cs0[:, 2, :250],
                                 ACTF.Abs_reciprocal_sqrt, scale=ISCL2)
            pnT_loc = persist_pool.tile([128, ND, SH], FP8, tag="pnT_loc")
            for d in range(ND):
                nc.vector.tensor_tensor(pnT_loc[:, d, :], pst[:, d, :],
                                        inv16[:, :], op=ALU.mult)

            # ---- AllGather the fp8 shards via DRAM bounce ------------------
            ag_in = dram_pool.tile([128, ND * SH], FP8, tag="ag_in")
            ag_out = dram_pool.tile([N_CORES * 128, ND * SH], FP8,
                                    tag="ag_out")
            pl_flat = pnT_loc[:, :, :].rearrange("p d c -> p (d c)")
            nc.gpsimd.dma_start(ag_in[:, :], pl_flat)
            nc.gpsimd.collective_compute(
                "AllGather",
                ALU.bypass,
                replica_groups=[list(range(N_CORES))],
                ins=[ag_in.opt()],
                outs=[ag_out.opt()],
            )
            ag_r = ag_out[:, :].rearrange("(s p) (d c) -> s p d c",
                                          p=128, d=ND)
            for s in range(N_CORES):
                nc.sync.dma_start(pnT[:, :, s * SH:(s + 1) * SH], ag_r[s])

            # ---- scalar params: c0 = -|ds|/temp, c1 = c0/C -----------------
            stile = const_pool.tile([1, 2], F32, tag="stile")
            nc.sync.dma_start(stile[:, :], s_h[:, :])
            cv = const_pool.tile([1, 2], F32, tag="cvals")
            tmp = const_pool.tile([1, 2], F32, tag="scaltmp")
            nc.vector.scalar_tensor_tensor(tmp[:, 0:1], stile[:, 0:1], -1.0,
                                           stile[:, 0:1], op0=ALU.mult,
                                           op1=ALU.max)
            nc.vector.reciprocal(tmp[:, 1:2], stile[:, 1:2])
            nc.vector.scalar_tensor_tensor(cv[:, 0:1], tmp[:, 0:1], -1.0,
                                           tmp[:, 1:2], op0=ALU.mult,
                                           op1=ALU.mult)
            nc.vector.tensor_scalar(cv[:, 1:2], cv[:, 0:1], 1.0 / C, None,
                                    op0=ALU.mult)
            ps_s0 = psum_pool.tile([128, GRP, 512], F32, tag="ps4",
                                   name="ps_scal")
            nc.tensor.matmul(ps_s0[:, 0, :2], ones_f[:, :], cv[:, :],
                             start=True, stop=True)
            nc.vector.tensor_copy(cb[:, :], ps_s0[:, 0, :2])

            # ---- feature prep ---------------------------------------------
            f_r = f_h[:, :].rearrange("(g p) d -> g p d", p=128)
            for g in range(NB):
                fst = fstage_pool.tile([128, D], F32, tag="fst")
                nc.sync.dma_start(fst[:, :], f_r[g])
                fss = norm_pool.tile([128, 1], F32, tag="fss",
                                     name=f"fss_{g}")
                finv = norm_pool.tile([128, 1], F32, tag="finv",
                                      name=f"finv_{g}")
                fsq = bf_pool.tile([128, D], BF16, tag="bfc",
                                   name=f"fsq_{g}")
                nc.vector.scalar_tensor_tensor(
                    fsq[:, :], fst[:, :], 1.0, fst[:, :],
                    op0=ALU.mult, op1=ALU.mult, accum_out=fss[:, 0:1])
                nc.scalar.activation(finv[:, :], fss[:, :],
                                     ACTF.Abs_reciprocal_sqrt, scale=ISCL2)
                fbf = bf_pool.tile([128, D], BF16, tag="bfc",
                                   name=f"fbf_{g}")
                nc.vector.tensor_scalar(fbf[:, :], fst[:, :], finv[:, 0:1],
                                        None, op0=ALU.mult)
                ps_t0 = psum_pool.tile([128, GRP, 512], F32, tag="ps4",
                                       name=f"ps_t_{g}")
                ps_t = ps_t0[:, 0, :].bitcast(BF16)[:, :D].rearrange(
                    "p (d c) -> p d c", d=ND)
                for d in range(ND):
                    nc.tensor.transpose(ps_t[:, d, :],
                                        fbf[:, d * 128:(d + 1) * 128],
                                        ident[:, :])
                nc.vector.tensor_copy(
                    fnT[:, :, g * 128:(g + 1) * 128], ps_t[:, :, :])

            # ---- main loop -------------------------------------------------
            for i in range(NB):
                rs = rs_pool.tile([128, NG], F32, tag="rs")
                dist = dist_pool.tile([128, C], BF16, tag="dist")
                for g in range(NG):
                    ps = psum_pool.tile([128, GRP, 512], F32, tag="ps4",
                                        name=f"ps_{i}_{g}")
                    for p in range(ND // 2):
                        for k in range(GRP):
                            c0 = (g * GRP + k) * CCH
                            nc.tensor.matmul(
                                ps[:, k, :CCH],
                                fnT[:, 2 * p:2 * p + 2,
                                    i * 128:(i + 1) * 128],
                                pnT[:, 2 * p:2 * p + 2, c0:c0 + CCH],
                                start=(p == 0), stop=(p == ND // 2 - 1),
                                perf_mode=DR)
                    dv = dist[:, g * GRP * CCH:(g + 1) * GRP * CCH].rearrange(
                        "p (k c) -> p k c", k=GRP)
                    nc.scalar.activation(
                        dv, ps[:, :, :CCH],
                        ACTF.Sqrt, bias=1.0, scale=-ISCL2,
                        accum_out=rs[:, g:g + 1])
                rsum = norm_pool.tile([128, 1], F32, tag="rsum")
                bvec = norm_pool.tile([128, 1], F32, tag="bvec")
                nc.vector.reduce_sum(rsum[:, :], rs[:, :],
                                     axis=mybir.AxisListType.X)
                nc.vector.tensor_scalar(bvec[:, :], rsum[:, :], cb[:, 1:2],
                                        None, op0=ALU.mult)
                for g in range(NG):
                    ob = ob_pool.tile([128, GRP * CCH], BF16, tag="ob",
                                      name=f"ob_{i}_{g}")
                    eng = nc.vector if g < 4 else nc.gpsimd
                    eng.tensor_scalar(ob[:, :],
                                      dist[:, g * GRP * CCH:(g + 1) * GRP * CCH],
                                      cb[:, 0:1], bvec[:, 0:1],
                                      op0=ALU.mult, op1=ALU.add)
                    nc.sync.dma_start(
                        o_h[i * 128:(i + 1) * 128,
                            g * GRP * CCH:(g + 1) * GRP * CCH],
                        ob[:, :])

    nc.compile()
    return nc


_CACHE = {}


def _get_nc():
    if "nc" not in _CACHE:
        _CACHE["nc"] = build_nc()
    return _CACHE["nc"]


def make_in_maps(features, prototypes, distance_scale, temperature):
    f = np.ascontiguousarray(np.asarray(features, dtype=np.float32))
    pt = np.asarray(prototypes, dtype=np.float32).T
    s = np.array([[np.float32(np.asarray(distance_scale).reshape(-1)[0]),
                   np.float32(np.asarray(temperature).reshape(-1)[0])]],
                 dtype=np.float32)
    return [
        {"f": f[i * BPC:(i + 1) * BPC],
         "pt": np.ascontiguousarray(pt[:, i * SH:(i + 1) * SH]),
         "s": s}
        for i in range(N_CORES)
    ]


def run(features, prototypes, distance_scale, temperature, **kwargs):
    nc = _get_nc()
    in_maps = make_in_maps(features, prototypes, distance_scale, temperature)
    res = run_bass_kernel_spmd(nc, in_maps, core_ids=list(range(N_CORES)),
                               **kwargs)
    out = np.concatenate(
        [np.asarray(res.results[i]["o"]).astype(np.float32)
         for i in range(N_CORES)], axis=0)
    return out, res


def kernel(features, prototypes, distance_scale, temperature):
    out, _ = run(features, prototypes, distance_scale, temperature)
    return out
